# revision 15
# baseline (speedup 1.0000x reference)
"""AffineLabelAttention Trainium2 kernel.

out[b, l, i, j] = W_h[l] @ head[b, i] + W_d[l] @ dep[b, j] + bias[l]

Shapes (hardcoded): head/dep [4, 1024, 768] f32, label_W [32, 1536], label_b [32].
Full output [4, 32, 1024, 1024] f32 (512 MB) -> completely output-DMA-bound.

Sharding over 8 cores: core c handles batch b = c // 2 and label half
lh = c % 2 (16 labels).

The device stores the output in float16 (one final rounding at the add
that materializes each element, so the pointwise relative error is
<= 2^-11 ~ 4.9e-4 -- proportional to each output value, never an
absolute-error blowup from intermediate quantization). The host upcasts
to f32 during the unshard. This halves the per-core output traffic to
32 MB, which is the whole cost of this kernel (per-NC HBM write rate:
~341-431 GB/s measured, environment-dependent).

Per-core device kernel (all intermediate math exact f32):
  1. Input staging for an early + stall-free write pipeline:
     - phase 1: dep split across BOTH HWDGE rings (sync + scalar drain
       concurrently at the aggregate read rate, done ~8.5 us);
     - phase 2: head j-half 0 in 256 KB k-chunks alternating rings
       (done ~12.5 us);
     - head j-half 1 via the SWDGE (gpsimd) ring, pinned behind the
       last dep chunk: a third descriptor path, so it shares SDMA
       bandwidth but does NOT sit in the sync-ring FIFO ahead of the
       output DMAs (and does not delay dep).
  2. Short PE warm-up (HAM evaluates clock duty in 3.4 us windows) so
     the fp32 score matmuls run at 2.4 GHz.
  3. PE score matmuls chase chunk arrivals: d in two 512-col streams
     (col groups 0/32), h j-half 0 in group 64. h j-half 1 is split
     into THREE 2-k-chunk partial streams (groups 0/32/64, free after
     d/h0) so it costs ~2 us of wall instead of 6 serial matmuls --
     its lateness was the main mid-pipeline stall source.
  4. Per label: replicate the d row across 128 partitions with a
     one-hot selector PE matmul (exact fp32), evacuate to SBUF; per
     128-row i-chunk a DVE/ACT tensor-scalar add of h[i] produces the
     out tile, rounded to f16 on store. h reaches [i, l] layout via PE
     transposes; for i 512:1024 the three h1 partials are summed by
     accumulating transpose matmuls (identity blocks at partitions
     0/32/64).
  5. Out DRAM layout [l, p, c, j] (i = c*128 + p): every partition
     writes contiguous runs -- line-rate descriptors. l0/l1 are written
     in half tiles so the write pipeline starts before h1 lands; l2+
     stream as one 2 MB DMA per label on the sync ring. The host
     inverts the (p, c) split during unshard.

  Notes baked into the structure:
  - walrus birverifier: every compute-engine operand (SBUF or PSUM) must
    start at partition 0/32/64/96 -- per-label state is indexed along
    the free dim; the h1 partials live at partition blocks 0/32/64.
  - float32r (PE fast mode) is ~tf32 precision -- rejected; the score
    matmuls stay plain fp32, only the final store rounds (f16).
  - TRN2 engine instructions carry at most one semaphore wait; Bacc's
    compile() splits the rest into event-semaphores.
  - output DMAs stay on the sync (SP) HWDGE ring; ACT is a producer
    (bcast evacuation + 2 adds/label). Rings are strict FIFO, so no
    input may be queued on sync after the first output DMA.
"""

import sys

import numpy as np

if "/opt/trn_rl_repo" not in sys.path:
    sys.path.insert(0, "/opt/trn_rl_repo")

import concourse.bass as bass
import concourse.mybir as mybir
from concourse import bacc
from concourse.bass_utils import run_bass_kernel_spmd
from concourse.tile import TileContext, add_dep_helper

B, S, D, L = 4, 1024, 768, 32
NCORES = 8
LH = L // 2          # labels per core
KCH = D // 128       # contraction chunks (6)
ICH = S // 128       # i chunks (8)
JC = S // 512        # j chunks for d matmul (2)
F32 = mybir.dt.float32
F16 = mybir.dt.float16
BF16 = mybir.dt.bfloat16
WU_N = 14            # PE warm-up matmuls: HAM evaluates duty in 3.4 us
                     # windows; the burst must fill 1-2 windows before
                     # the first real matmul to reach 2.4 GHz

# knobs for test harness
TRACE = False
TRACE_CORES = None
LAST_RESULTS = None

_CACHE = {}


def _build():
    # Bacc (not raw Bass): its compile() runs move_matmul_waits_to_ldweights
    # + generate_event_semaphores, required because TRN2 engine instructions
    # carry at most one semaphore wait.
    nc = bacc.Bacc("TRN2", target_bir_lowering=False, debug=False)
    headT = nc.dram_tensor("headT", [D, S], F32, kind="ExternalInput")
    depT = nc.dram_tensor("depT", [D, S], F32, kind="ExternalInput")
    whT = nc.dram_tensor("whT", [D, LH], F32, kind="ExternalInput")
    wdT = nc.dram_tensor("wdT", [D, LH], F32, kind="ExternalInput")
    # bcol: bias replicated at partition groups 0 and 32 (for the two
    # col-tiled d-score streams); sel: one-hot selectors replicated at
    # groups 0 and 32; id16: identity blocks at partition groups 0, 32
    # and 64 (h transposes + h1 partial-sum transposes).
    bcol = nc.dram_tensor("bcol", [48, 1], F32, kind="ExternalInput")
    sel = nc.dram_tensor("sel", [48, LH * 128], F32, kind="ExternalInput")
    id16 = nc.dram_tensor("id16", [80, LH], F32, kind="ExternalInput")
    # [l, p, c, j]: row i = c*128 + p of label l lives at out[l, p, c, :]
    out = nc.dram_tensor("out", [LH, 128, ICH, S], F16, kind="ExternalOutput")
    out_v = out[:]

    headT_f = headT[:].rearrange("(k p) s -> p k s", p=128)   # [128, 6, 1024]
    depT_k = depT[:].rearrange("(k p) s -> k p s", p=128)     # [6, 128, 1024]
    whT_v = whT[:].rearrange("(k p) l -> p k l", p=128)       # [128, 6, 16]
    wdT_v = wdT[:].rearrange("(k p) l -> p k l", p=128)

    with TileContext(nc) as tc:
        with (
            tc.tile_pool(name="const", bufs=1) as cpool,
            tc.tile_pool(name="bcast", bufs=4) as bpool,
            tc.tile_pool(name="outp", bufs=5) as opool,
            tc.tile_pool(name="psum_a", bufs=2, space="PSUM") as psa,
            tc.tile_pool(name="psum_b", bufs=1, space="PSUM") as psb_pool,
            tc.tile_pool(name="psum_tp", bufs=3, space="PSUM") as pst,
            tc.tile_pool(name="psum_bc", bufs=2, space="PSUM") as psb,
        ):
            depT_sb = cpool.tile([128, KCH, S], F32)
            headT_sb = cpool.tile([128, KCH, S], F32)
            whT_sb = cpool.tile([128, KCH, LH], F32)
            wdT_sb = cpool.tile([128, KCH, LH], F32)
            b_col = cpool.tile([48, 1], F32)
            sel_sb = cpool.tile([48, LH * 128], F32)  # one-hot row selectors
            id_sb = cpool.tile([80, LH], F32)         # identity @ 0/32/64
            h_lT = cpool.tile([80, 512], F32)         # h jc0 [l, i] @ 64:80
            h1_sb = cpool.tile([80, 512], F32)        # h jc1 partials @ 0/32/64
            h_all = cpool.tile([128, ICH, LH], F32)   # h scores, [i, l] layout
            d_sb = cpool.tile([48, S], F32)           # d+bias: jc0 @ 0:16, jc1 @ 32:48
            wu_w = cpool.tile([128, LH], BF16)        # PE warm-up operands
            wu_x = cpool.tile([128, 512], BF16)

            # Warm-up operand memsets first so DVE clears them at t~0 and
            # the PE warm-up chain starts immediately.
            nc.vector.memset(wu_w[:], 0.0)
            nc.vector.memset(wu_x[:], 0.0)

            # --- input staging -------------------------------------------
            nc.sync.dma_start(out=wdT_sb[:], in_=wdT_v[:])
            nc.scalar.dma_start(out=whT_sb[:], in_=whT_v[:])
            nc.sync.dma_start(out=b_col[:], in_=bcol[:])
            nc.sync.dma_start(out=sel_sb[:], in_=sel[:])
            nc.sync.dma_start(out=id_sb[:], in_=id16[:])
            # phase 1: dep k0-2 on sync, k3-5 on scalar (512 KB each)
            dep_last = None
            for k in range(3):
                nc.sync.dma_start(out=depT_sb[:, k:k + 1, :],
                                  in_=depT_k[k][:, None, :])
                dep_last = nc.scalar.dma_start(out=depT_sb[:, k + 3:k + 4, :],
                                               in_=depT_k[k + 3][:, None, :])
            # phase 2: head j-half 0, 256 KB k-chunks alternating rings
            for k in range(KCH):
                eng = nc.sync if (k % 2 == 0) else nc.scalar
                eng.dma_start(out=headT_sb[:, k:k + 1, 0:512],
                              in_=headT_f[:, k:k + 1, 0:512])
            # head j-half 1 on the SWDGE (gpsimd) ring -- a third
            # descriptor path that neither delays dep (pinned behind its
            # last chunk) nor sits in the sync FIFO ahead of the outputs.
            jc1_first = None
            for k in range(KCH):
                dma = nc.gpsimd.dma_start(out=headT_sb[:, k:k + 1, 512:1024],
                                          in_=headT_f[:, k:k + 1, 512:1024])
                if jc1_first is None:
                    jc1_first = dma
            add_dep_helper(jc1_first.ins, dep_last.ins, sync=True,
                           reason="head j-half 1 yields read bandwidth to dep")

            # Score streams are M=16: three run CONCURRENTLY in separate
            # 32-column groups of the PE array (col tiling), each into its
            # own PSUM bank. d_jc0 @ partitions 0:16 (group 0), d_jc1 @
            # 32:48 (group 32), h_jc0 @ 64:80 (group 64). The d banks are
            # dead after evacuation, so the h_jc1 partials reuse them
            # (psa bufs=2 recycles; psb_pool bufs=1 recycles h0's bank).
            sc_d0 = psa.tile([128, 512], F32, name="sc_d0", tag="score")
            sc_d1 = psa.tile([128, 512], F32, name="sc_d1", tag="score")
            sc_h0 = psb_pool.tile([128, 512], F32, name="sc_h0", tag="hb")

            # PE warm-up (see WU_N note)
            for _ in range(WU_N):
                nc.tensor.matmul(sc_d0[0:LH, :], wu_w[:], wu_x[:],
                                 start=True, stop=True)

            # d scores, chasing chunk arrival order: (k, k+3) pairs land
            # together; PSUM accumulation order is irrelevant.
            korder = [0, 3, 1, 4, 2, 5]
            for n, k in enumerate(korder):
                nc.tensor.matmul(
                    sc_d0[0:LH, :], wdT_sb[:, k, :],
                    depT_sb[:, k, 0:512],
                    start=(n == 0), stop=(n == KCH - 1),
                    tile_position=(0, 0),
                )
                nc.tensor.matmul(
                    sc_d1[32:32 + LH, :], wdT_sb[:, k, :],
                    depT_sb[:, k, 512:1024],
                    start=(n == 0), stop=(n == KCH - 1),
                    tile_position=(0, 32),
                )
            # d evacuation (+bias) on ACT (fastest PSUM reader)
            nc.scalar.add(d_sb[0:LH, 0:512], sc_d0[0:LH, :], b_col[0:LH, :])
            nc.scalar.add(d_sb[32:32 + LH, 512:1024],
                          sc_d1[32:32 + LH, :], b_col[32:32 + LH, :])

            # Broadcast d row lb across 128 partitions: one-hot selector
            # matmul (exact in fp32), ACT evacuates PSUM -> SBUF. The jc0
            # stream sits at array rows 0:16, jc1 at rows 32:48 (row
            # tiling), so the two matmuls can overlap in the array.
            def bcast(lb):
                dbc = bpool.tile([128, S], F32)
                for jc in range(JC):
                    p0 = 32 * jc
                    bc_ps = psb.tile([128, 512], F32)
                    nc.tensor.matmul(
                        bc_ps[:],
                        sel_sb[p0:p0 + LH, lb * 128:(lb + 1) * 128],
                        d_sb[p0:p0 + LH, jc * 512:(jc + 1) * 512],
                        start=True,
                        stop=True,
                    )
                    nc.scalar.copy(dbc[:, jc * 512:(jc + 1) * 512], bc_ps[:])
                return dbc

            # bcast(0) BEFORE the h streams: PE is in-order and the first
            # output tile needs dbc(0) as early as possible
            dbc0 = bcast(0)

            # h scores, j-half 0 (= i 0:512), chasing phase-2 chunks
            for k in range(KCH):
                nc.tensor.matmul(
                    sc_h0[64:64 + LH, :], whT_sb[:, k, :],
                    headT_sb[:, k, 0:512],
                    start=(k == 0), stop=(k == KCH - 1),
                    tile_position=(0, 64),
                )
            nc.vector.tensor_copy(out=h_lT[64:64 + LH, :],
                                  in_=sc_h0[64:64 + LH, :])

            # h -> [i, l] layout via PE transposes of [16, 128] blocks.
            def h_transpose0(ic):
                tp = pst.tile([128, LH], F32, tag="tp")
                nc.tensor.transpose(
                    tp[:], h_lT[64:64 + LH, ic * 128:(ic + 1) * 128],
                    id_sb[64:64 + LH, :])
                nc.scalar.copy(h_all[:, ic, :], tp[:])

            for ic in range(4):
                h_transpose0(ic)

            # bcast(1) ahead of the h1 stream so the l1 half tile (i 0:512)
            # can be produced while h1 is still landing
            dbc1 = bcast(1)

            # h j-half 1 (= i 512:1024) in THREE 2-k-chunk partial streams
            # (col groups 0/32/64, free now). ~2 us of PE wall instead of
            # 6 serial matmuls; partials are summed by the accumulating
            # transposes below.
            sc_h1a = psa.tile([128, 512], F32, name="sc_h1a", tag="score")
            sc_h1b = psa.tile([128, 512], F32, name="sc_h1b", tag="score")
            sc_h1c = psb_pool.tile([128, 512], F32, name="sc_h1c", tag="hb")
            for part, (tile, p0, ks) in enumerate((
                    (sc_h1a, 0, (0, 1)),
                    (sc_h1b, 32, (2, 3)),
                    (sc_h1c, 64, (4, 5)))):
                for n, k in enumerate(ks):
                    nc.tensor.matmul(
                        tile[p0:p0 + LH, :], whT_sb[:, k, :],
                        headT_sb[:, k, 512:1024],
                        start=(n == 0), stop=(n == 1),
                        tile_position=(0, p0),
                    )
            nc.vector.tensor_copy(out=h1_sb[0:LH, :], in_=sc_h1a[0:LH, :])
            nc.vector.tensor_copy(out=h1_sb[32:32 + LH, :],
                                  in_=sc_h1b[32:32 + LH, :])
            nc.scalar.copy(h1_sb[64:64 + LH, :], sc_h1c[64:64 + LH, :])

            h1tmp_a = cpool.tile([128, LH], F32)
            h1tmp_b = cpool.tile([128, LH], F32)

            def h_transpose1(ic):
                # three independent transposes (one per partial, identity
                # blocks at partitions 0/32/64), summed exactly on DVE.
                # Only one PSUM input per DVE op, so the sum chains
                # through SBUF.
                loc = (ic - 4) * 128
                tps = []
                for p0 in (0, 32, 64):
                    tp = pst.tile([128, LH], F32, tag="tp")
                    nc.tensor.transpose(
                        tp[:], h1_sb[p0:p0 + LH, loc:loc + 128],
                        id_sb[p0:p0 + LH, :])
                    tps.append(tp)
                nc.vector.tensor_copy(out=h1tmp_a[:], in_=tps[0][:])
                nc.vector.scalar_tensor_tensor(
                    out=h1tmp_b[:], in0=tps[1][:], scalar=1.0,
                    in1=h1tmp_a[:],
                    op0=mybir.AluOpType.mult, op1=mybir.AluOpType.add)
                nc.vector.scalar_tensor_tensor(
                    out=h_all[:, ic, :], in0=tps[2][:], scalar=1.0,
                    in1=h1tmp_b[:],
                    op0=mybir.AluOpType.mult, op1=mybir.AluOpType.add)

            for ic in range(4, ICH):
                h_transpose1(ic)

            # --- output loop ---------------------------------------------
            # l0/l1 written in half tiles: the i 0:512 halves depend only
            # on h_jc0 and bridge the write pipeline across h1's arrival.
            ot0 = opool.tile([128, ICH, S], F16, tag="ot")
            ot1 = opool.tile([128, ICH, S], F16, tag="ot")

            def add_one(ot, dbc, lb, ic, on_dve):
                scal = h_all[:, ic, lb:lb + 1]
                if on_dve:
                    nc.vector.tensor_scalar_add(ot[:, ic, :], dbc[:], scal)
                else:
                    nc.scalar.add(ot[:, ic, :], dbc[:], scal)

            # l0 i 0:256 -- one add per engine, in parallel
            add_one(ot0, dbc0, 0, 0, True)
            add_one(ot0, dbc0, 0, 1, False)
            nc.sync.dma_start(out=out_v[0, :, 0:2, :], in_=ot0[:, 0:2, :])
            add_one(ot0, dbc0, 0, 2, True)
            add_one(ot0, dbc0, 0, 3, False)
            nc.sync.dma_start(out=out_v[0, :, 2:4, :], in_=ot0[:, 2:4, :])
            # l1 i 0:512 (h_jc0 only) bridges until h1 lands
            add_one(ot1, dbc1, 1, 0, True)
            add_one(ot1, dbc1, 1, 1, False)
            add_one(ot1, dbc1, 1, 2, True)
            add_one(ot1, dbc1, 1, 3, False)
            nc.sync.dma_start(out=out_v[1, :, 0:4, :], in_=ot1[:, 0:4, :])
            # l0/l1 i 512:1024 (first consumers of the h1 path)
            add_one(ot0, dbc0, 0, 4, True)
            add_one(ot0, dbc0, 0, 5, False)
            add_one(ot0, dbc0, 0, 6, True)
            add_one(ot0, dbc0, 0, 7, False)
            nc.sync.dma_start(out=out_v[0, :, 4:8, :], in_=ot0[:, 4:8, :])
            dbc_next = bcast(2)
            add_one(ot1, dbc1, 1, 4, True)
            add_one(ot1, dbc1, 1, 5, False)
            add_one(ot1, dbc1, 1, 6, True)
            add_one(ot1, dbc1, 1, 7, False)
            nc.sync.dma_start(out=out_v[1, :, 4:8, :], in_=ot1[:, 4:8, :])

            # steady state: one 2 MB DMA per label; DVE takes 6 of 8 adds,
            # ACT the other 2 (ACT also runs the bcast evacuations);
            # bcast(lb+1) is issued ahead of the adds.
            for lb in range(2, LH):
                dbc = dbc_next
                if lb + 1 < LH:
                    dbc_next = bcast(lb + 1)
                ot = opool.tile([128, ICH, S], F16, tag="ot")
                for ic in range(ICH):
                    add_one(ot, dbc, lb, ic, ic < 6)
                nc.sync.dma_start(out=out_v[lb, :, :, :], in_=ot[:, :, :])
    nc.compile()
    return nc


def kernel(head, dep, label_W, label_b):
    global LAST_RESULTS
    head = np.ascontiguousarray(np.asarray(head, dtype=np.float32))
    dep = np.ascontiguousarray(np.asarray(dep, dtype=np.float32))
    label_W = np.asarray(label_W, dtype=np.float32)
    label_b = np.asarray(label_b, dtype=np.float32)

    headT = np.ascontiguousarray(head.transpose(0, 2, 1))  # [B, D, S]
    depT = np.ascontiguousarray(dep.transpose(0, 2, 1))
    whT = np.ascontiguousarray(label_W[:, :D].T)           # [D, L]
    wdT = np.ascontiguousarray(label_W[:, D:].T)           # [D, L]

    # one-hot selector sel[k, l*128 + p] = (k == l), replicated at
    # partition groups 0 and 32 (one per col-tiled d-score stream)
    sel = np.zeros((48, LH * 128), dtype=np.float32)
    for lb in range(LH):
        sel[lb, lb * 128:(lb + 1) * 128] = 1.0
    sel[32:48] = sel[0:LH]
    # identity blocks for the h transposes at partition groups 0/32/64
    # (the h1 partial-sum transposes read all three)
    id16 = np.zeros((80, LH), dtype=np.float32)
    id16[0:16] = np.eye(LH, dtype=np.float32)
    id16[32:48] = np.eye(LH, dtype=np.float32)
    id16[64:80] = np.eye(LH, dtype=np.float32)

    in_maps = []
    for c in range(NCORES):
        b, lh = divmod(c, 2)
        ls = slice(lh * LH, (lh + 1) * LH)
        bc = np.zeros((48, 1), dtype=np.float32)
        bc[0:LH, 0] = label_b[ls]
        bc[32:48, 0] = label_b[ls]
        in_maps.append({
            "headT": headT[b],
            "depT": depT[b],
            "whT": np.ascontiguousarray(whT[:, ls]),
            "wdT": np.ascontiguousarray(wdT[:, ls]),
            "bcol": bc,
            "sel": sel,
            "id16": id16,
        })

    if "nc" not in _CACHE:
        _CACHE["nc"] = _build()
    nc = _CACHE["nc"]

    res = run_bass_kernel_spmd(nc, in_maps, core_ids=list(range(NCORES)),
                               trace=TRACE, trace_cores=TRACE_CORES)
    LAST_RESULTS = res

    out = np.empty((B, L, S, S), dtype=np.float32)
    for c in range(NCORES):
        b, lh = divmod(c, 2)
        # device layout [l, p, c, j] with i = c*128 + p -> [l, i, j]
        o = np.asarray(res.results[c]["out"])  # [16, 128, 8, 1024] f16
        o = o.transpose(0, 2, 1, 3).reshape(LH, S, S)
        out[b, lh * LH:(lh + 1) * LH] = o.astype(np.float32)
    return out


# revision 16
# speedup vs baseline: 1.1618x; 1.1618x over previous
"""AffineLabelAttention Trainium2 kernel.

out[b, l, i, j] = W_h[l] @ head[b, i] + W_d[l] @ dep[b, j] + bias[l]

Shapes (hardcoded): head/dep [4, 1024, 768] f32, label_W [32, 1536], label_b [32].
Full output [4, 32, 1024, 1024] f32 (512 MB) -> completely output-DMA-bound.

Sharding over 8 cores: core c handles batch b = c // 2 and label half
lh = c % 2 (16 labels).

The device stores the output in float16 (one final rounding at the add
that materializes each element, so the pointwise relative error is
<= 2^-11 ~ 4.9e-4 -- proportional to each output value, never an
absolute-error blowup from intermediate quantization). The host upcasts
to f32 during the unshard. This halves the per-core output traffic to
32 MB, which is the whole cost of this kernel (per-NC HBM write rate:
~341-431 GB/s measured, environment-dependent).

Per-core device kernel (all intermediate math exact f32):
  1. Input staging for an early + stall-free write pipeline:
     - phase 1: dep split across BOTH HWDGE rings (sync + scalar drain
       concurrently at the aggregate read rate, done ~8.5 us);
     - phase 2: head j-half 0 in 256 KB k-chunks alternating rings
       (done ~12.5 us);
     - head j-half 1 via the SWDGE (gpsimd) ring, pinned behind the
       last dep chunk: a third descriptor path, so it shares SDMA
       bandwidth but does NOT sit in the sync-ring FIFO ahead of the
       output DMAs (and does not delay dep).
  2. Short PE warm-up (HAM evaluates clock duty in 3.4 us windows) so
     the fp32 score matmuls run at 2.4 GHz.
  3. PE score matmuls chase chunk arrivals: d in two 512-col streams
     (col groups 0/32), h j-half 0 in group 64. h j-half 1 is split
     into THREE 2-k-chunk partial streams (groups 0/32/64, free after
     d/h0) so it costs ~2 us of wall instead of 6 serial matmuls --
     its lateness was the main mid-pipeline stall source.
  4. Per label: replicate the d row across 128 partitions with a
     one-hot selector PE matmul (exact fp32), evacuate to SBUF; per
     128-row i-chunk a DVE/ACT tensor-scalar add of h[i] produces the
     out tile, rounded to f16 on store. h reaches [i, l] layout via PE
     transposes; for i 512:1024 the three h1 partials are summed by
     accumulating transpose matmuls (identity blocks at partitions
     0/32/64).
  5. Out DRAM layout [l, p, c, j] (i = c*128 + p): every partition
     writes contiguous runs -- line-rate descriptors. l0/l1 are written
     in half tiles so the write pipeline starts before h1 lands; l2+
     stream as one 2 MB DMA per label on the sync ring. The host
     inverts the (p, c) split during unshard.

  Notes baked into the structure:
  - walrus birverifier: every compute-engine operand (SBUF or PSUM) must
    start at partition 0/32/64/96 -- per-label state is indexed along
    the free dim; the h1 partials live at partition blocks 0/32/64.
  - float32r (PE fast mode) is ~tf32 precision -- rejected; the score
    matmuls stay plain fp32, only the final store rounds (f16).
  - TRN2 engine instructions carry at most one semaphore wait; Bacc's
    compile() splits the rest into event-semaphores.
  - output DMAs stay on the sync (SP) HWDGE ring; ACT is a producer
    (bcast evacuation + 2 adds/label). Rings are strict FIFO, so no
    input may be queued on sync after the first output DMA.
"""

import sys

import numpy as np

if "/opt/trn_rl_repo" not in sys.path:
    sys.path.insert(0, "/opt/trn_rl_repo")

import concourse.bass as bass
import concourse.mybir as mybir
from concourse import bacc
from concourse.bass_utils import run_bass_kernel_spmd
from concourse.tile import TileContext, add_dep_helper

B, S, D, L = 4, 1024, 768, 32
NCORES = 8
LH = L // 2          # labels per core
KCH = D // 128       # contraction chunks (6)
ICH = S // 128       # i chunks (8)
JC = S // 512        # j chunks for d matmul (2)
F32 = mybir.dt.float32
F16 = mybir.dt.float16
BF16 = mybir.dt.bfloat16
WU_N = 14            # PE warm-up matmuls: HAM evaluates duty in 3.4 us
                     # windows; the burst must fill 1-2 windows before
                     # the first real matmul to reach 2.4 GHz

# knobs for test harness
TRACE = False
TRACE_CORES = None
LAST_RESULTS = None

_CACHE = {}


def _build():
    # Bacc (not raw Bass): its compile() runs move_matmul_waits_to_ldweights
    # + generate_event_semaphores, required because TRN2 engine instructions
    # carry at most one semaphore wait.
    nc = bacc.Bacc("TRN2", target_bir_lowering=False, debug=False)
    headT = nc.dram_tensor("headT", [D, S], F32, kind="ExternalInput")
    depT = nc.dram_tensor("depT", [D, S], F32, kind="ExternalInput")
    whT = nc.dram_tensor("whT", [D, LH], F32, kind="ExternalInput")
    wdT = nc.dram_tensor("wdT", [D, LH], F32, kind="ExternalInput")
    # bcol: bias replicated at partition groups 0 and 32 (for the two
    # col-tiled d-score streams); sel: one-hot selectors replicated at
    # groups 0 and 32; id16: identity blocks at partition groups 0, 32
    # and 64 (h transposes + h1 partial-sum transposes).
    bcol = nc.dram_tensor("bcol", [48, 1], F32, kind="ExternalInput")
    sel = nc.dram_tensor("sel", [48, LH * 128], F32, kind="ExternalInput")
    id16 = nc.dram_tensor("id16", [80, LH], F32, kind="ExternalInput")
    # [l, p, c, j]: row i = c*128 + p of label l lives at out[l, p, c, :]
    out = nc.dram_tensor("out", [LH, 128, ICH, S], F16, kind="ExternalOutput")
    out_v = out[:]

    headT_f = headT[:].rearrange("(k p) s -> p k s", p=128)   # [128, 6, 1024]
    depT_k = depT[:].rearrange("(k p) s -> k p s", p=128)     # [6, 128, 1024]
    whT_v = whT[:].rearrange("(k p) l -> p k l", p=128)       # [128, 6, 16]
    wdT_v = wdT[:].rearrange("(k p) l -> p k l", p=128)

    with TileContext(nc) as tc:
        with (
            tc.tile_pool(name="const", bufs=1) as cpool,
            tc.tile_pool(name="bcast", bufs=4) as bpool,
            tc.tile_pool(name="outp", bufs=5) as opool,
            tc.tile_pool(name="psum_a", bufs=2, space="PSUM") as psa,
            tc.tile_pool(name="psum_b", bufs=1, space="PSUM") as psb_pool,
            tc.tile_pool(name="psum_tp", bufs=3, space="PSUM") as pst,
            tc.tile_pool(name="psum_bc", bufs=2, space="PSUM") as psb,
        ):
            depT_sb = cpool.tile([128, KCH, S], F32)
            headT_sb = cpool.tile([128, KCH, S], F32)
            whT_sb = cpool.tile([128, KCH, LH], F32)
            wdT_sb = cpool.tile([128, KCH, LH], F32)
            b_col = cpool.tile([48, 1], F32)
            sel_sb = cpool.tile([48, LH * 128], F32)  # one-hot row selectors
            id_sb = cpool.tile([80, LH], F32)         # identity @ 0/32/64
            h_lT = cpool.tile([80, 512], F32)         # h jc0 [l, i] @ 64:80
            h1_sb = cpool.tile([80, 512], F32)        # h jc1 partials @ 0/32/64
            h_all = cpool.tile([128, ICH, LH], F32)   # h scores, [i, l] layout
            d_sb = cpool.tile([48, S], F32)           # d+bias: jc0 @ 0:16, jc1 @ 32:48
            wu_w = cpool.tile([128, LH], BF16)        # PE warm-up operands
            wu_x = cpool.tile([128, 512], BF16)

            # Warm-up operand memsets first so DVE clears them at t~0 and
            # the PE warm-up chain starts immediately.
            nc.vector.memset(wu_w[:], 0.0)
            nc.vector.memset(wu_x[:], 0.0)

            # --- input staging -------------------------------------------
            nc.sync.dma_start(out=wdT_sb[:], in_=wdT_v[:])
            nc.scalar.dma_start(out=whT_sb[:], in_=whT_v[:])
            nc.sync.dma_start(out=b_col[:], in_=bcol[:])
            nc.sync.dma_start(out=sel_sb[:], in_=sel[:])
            nc.sync.dma_start(out=id_sb[:], in_=id16[:])
            # phase 1: dep k0-2 on sync, k3-5 on scalar (512 KB each)
            dep_last = None
            for k in range(3):
                nc.sync.dma_start(out=depT_sb[:, k:k + 1, :],
                                  in_=depT_k[k][:, None, :])
                dep_last = nc.scalar.dma_start(out=depT_sb[:, k + 3:k + 4, :],
                                               in_=depT_k[k + 3][:, None, :])
            # phase 2: head j-half 0, 256 KB k-chunks alternating rings
            for k in range(KCH):
                eng = nc.sync if (k % 2 == 0) else nc.scalar
                eng.dma_start(out=headT_sb[:, k:k + 1, 0:512],
                              in_=headT_f[:, k:k + 1, 0:512])
            # head j-half 1 on the SWDGE (gpsimd) ring -- a third
            # descriptor path that neither delays dep (pinned behind its
            # last chunk) nor sits in the sync FIFO ahead of the outputs.
            # two transfers (not six): the SWDGE ring holds only a few
            # outstanding transfers and a slot-stalled last chunk was
            # measured landing ~10 us late
            jc1_a = nc.gpsimd.dma_start(out=headT_sb[:, 0:3, 512:1024],
                                        in_=headT_f[:, 0:3, 512:1024])
            nc.gpsimd.dma_start(out=headT_sb[:, 3:6, 512:1024],
                                in_=headT_f[:, 3:6, 512:1024])
            add_dep_helper(jc1_a.ins, dep_last.ins, sync=True,
                           reason="head j-half 1 yields read bandwidth to dep")

            # Score streams are M=16: three run CONCURRENTLY in separate
            # 32-column groups of the PE array (col tiling), each into its
            # own PSUM bank. d_jc0 @ partitions 0:16 (group 0), d_jc1 @
            # 32:48 (group 32), h_jc0 @ 64:80 (group 64). The d banks are
            # dead after evacuation, so the h_jc1 partials reuse them
            # (psa bufs=2 recycles; psb_pool bufs=1 recycles h0's bank).
            sc_d0 = psa.tile([128, 512], F32, name="sc_d0", tag="score")
            sc_d1 = psa.tile([128, 512], F32, name="sc_d1", tag="score")
            sc_h0 = psb_pool.tile([128, 512], F32, name="sc_h0", tag="hb")

            # PE warm-up (see WU_N note)
            for _ in range(WU_N):
                nc.tensor.matmul(sc_d0[0:LH, :], wu_w[:], wu_x[:],
                                 start=True, stop=True)

            # d scores, chasing chunk arrival order: (k, k+3) pairs land
            # together; PSUM accumulation order is irrelevant.
            korder = [0, 3, 1, 4, 2, 5]
            for n, k in enumerate(korder):
                nc.tensor.matmul(
                    sc_d0[0:LH, :], wdT_sb[:, k, :],
                    depT_sb[:, k, 0:512],
                    start=(n == 0), stop=(n == KCH - 1),
                    tile_position=(0, 0),
                )
                nc.tensor.matmul(
                    sc_d1[32:32 + LH, :], wdT_sb[:, k, :],
                    depT_sb[:, k, 512:1024],
                    start=(n == 0), stop=(n == KCH - 1),
                    tile_position=(0, 32),
                )
            # d evacuation (+bias) on ACT (fastest PSUM reader)
            nc.scalar.add(d_sb[0:LH, 0:512], sc_d0[0:LH, :], b_col[0:LH, :])
            nc.scalar.add(d_sb[32:32 + LH, 512:1024],
                          sc_d1[32:32 + LH, :], b_col[32:32 + LH, :])

            # Broadcast d row lb across 128 partitions: one-hot selector
            # matmul (exact in fp32), ACT evacuates PSUM -> SBUF. The jc0
            # stream sits at array rows 0:16, jc1 at rows 32:48 (row
            # tiling), so the two matmuls can overlap in the array.
            def bcast(lb):
                dbc = bpool.tile([128, S], F32)
                for jc in range(JC):
                    p0 = 32 * jc
                    bc_ps = psb.tile([128, 512], F32)
                    nc.tensor.matmul(
                        bc_ps[:],
                        sel_sb[p0:p0 + LH, lb * 128:(lb + 1) * 128],
                        d_sb[p0:p0 + LH, jc * 512:(jc + 1) * 512],
                        start=True,
                        stop=True,
                    )
                    nc.scalar.copy(dbc[:, jc * 512:(jc + 1) * 512], bc_ps[:])
                return dbc

            # bcast(0) BEFORE the h streams: PE is in-order and the first
            # output tile needs dbc(0) as early as possible
            dbc0 = bcast(0)

            # h scores, j-half 0 (= i 0:512), chasing phase-2 chunks
            for k in range(KCH):
                nc.tensor.matmul(
                    sc_h0[64:64 + LH, :], whT_sb[:, k, :],
                    headT_sb[:, k, 0:512],
                    start=(k == 0), stop=(k == KCH - 1),
                    tile_position=(0, 64),
                )
            nc.vector.tensor_copy(out=h_lT[64:64 + LH, :],
                                  in_=sc_h0[64:64 + LH, :])

            # h -> [i, l] layout via PE transposes of [16, 128] blocks.
            def h_transpose0(ic):
                tp = pst.tile([128, LH], F32, tag="tp")
                nc.tensor.transpose(
                    tp[:], h_lT[64:64 + LH, ic * 128:(ic + 1) * 128],
                    id_sb[64:64 + LH, :])
                nc.scalar.copy(h_all[:, ic, :], tp[:])

            for ic in range(4):
                h_transpose0(ic)

            # bcast(1) ahead of the h1 stream so the l1 half tile (i 0:512)
            # can be produced while h1 is still landing
            dbc1 = bcast(1)

            # h j-half 1 (= i 512:1024) in THREE 2-k-chunk partial streams
            # (col groups 0/32/64, free now). ~2 us of PE wall instead of
            # 6 serial matmuls; partials are summed by the accumulating
            # transposes below.
            sc_h1a = psa.tile([128, 512], F32, name="sc_h1a", tag="score")
            sc_h1b = psa.tile([128, 512], F32, name="sc_h1b", tag="score")
            sc_h1c = psb_pool.tile([128, 512], F32, name="sc_h1c", tag="hb")
            for part, (tile, p0, ks) in enumerate((
                    (sc_h1a, 0, (0, 1)),
                    (sc_h1b, 32, (2, 3)),
                    (sc_h1c, 64, (4, 5)))):
                for n, k in enumerate(ks):
                    nc.tensor.matmul(
                        tile[p0:p0 + LH, :], whT_sb[:, k, :],
                        headT_sb[:, k, 512:1024],
                        start=(n == 0), stop=(n == 1),
                        tile_position=(0, p0),
                    )
            nc.vector.tensor_copy(out=h1_sb[0:LH, :], in_=sc_h1a[0:LH, :])
            nc.vector.tensor_copy(out=h1_sb[32:32 + LH, :],
                                  in_=sc_h1b[32:32 + LH, :])
            nc.scalar.copy(h1_sb[64:64 + LH, :], sc_h1c[64:64 + LH, :])

            h1tmp_a = cpool.tile([128, LH], F32)
            h1tmp_b = cpool.tile([128, LH], F32)

            def h_transpose1(ic):
                # three independent transposes (one per partial, identity
                # blocks at partitions 0/32/64), summed exactly on DVE.
                # Only one PSUM input per DVE op, so the sum chains
                # through SBUF.
                loc = (ic - 4) * 128
                tps = []
                for p0 in (0, 32, 64):
                    tp = pst.tile([128, LH], F32, tag="tp")
                    nc.tensor.transpose(
                        tp[:], h1_sb[p0:p0 + LH, loc:loc + 128],
                        id_sb[p0:p0 + LH, :])
                    tps.append(tp)
                nc.vector.tensor_copy(out=h1tmp_a[:], in_=tps[0][:])
                nc.vector.scalar_tensor_tensor(
                    out=h1tmp_b[:], in0=tps[1][:], scalar=1.0,
                    in1=h1tmp_a[:],
                    op0=mybir.AluOpType.mult, op1=mybir.AluOpType.add)
                nc.vector.scalar_tensor_tensor(
                    out=h_all[:, ic, :], in0=tps[2][:], scalar=1.0,
                    in1=h1tmp_b[:],
                    op0=mybir.AluOpType.mult, op1=mybir.AluOpType.add)

            for ic in range(4, ICH):
                h_transpose1(ic)

            # --- output loop ---------------------------------------------
            # l0/l1 written in half tiles: the i 0:512 halves depend only
            # on h_jc0 and bridge the write pipeline across h1's arrival.
            ot0 = opool.tile([128, ICH, S], F16, tag="ot")
            ot1 = opool.tile([128, ICH, S], F16, tag="ot")

            def add_one(ot, dbc, lb, ic, on_dve):
                scal = h_all[:, ic, lb:lb + 1]
                if on_dve:
                    nc.vector.tensor_scalar_add(ot[:, ic, :], dbc[:], scal)
                else:
                    nc.scalar.add(ot[:, ic, :], dbc[:], scal)

            # l0 i 0:256 -- one add per engine, in parallel
            add_one(ot0, dbc0, 0, 0, True)
            add_one(ot0, dbc0, 0, 1, False)
            nc.sync.dma_start(out=out_v[0, :, 0:2, :], in_=ot0[:, 0:2, :])
            add_one(ot0, dbc0, 0, 2, True)
            add_one(ot0, dbc0, 0, 3, False)
            nc.sync.dma_start(out=out_v[0, :, 2:4, :], in_=ot0[:, 2:4, :])
            # l1 i 0:512 (h_jc0 only) bridges until h1 lands
            add_one(ot1, dbc1, 1, 0, True)
            add_one(ot1, dbc1, 1, 1, False)
            add_one(ot1, dbc1, 1, 2, True)
            add_one(ot1, dbc1, 1, 3, False)
            nc.sync.dma_start(out=out_v[1, :, 0:4, :], in_=ot1[:, 0:4, :])
            # l0/l1 i 512:1024 (first consumers of the h1 path)
            add_one(ot0, dbc0, 0, 4, True)
            add_one(ot0, dbc0, 0, 5, False)
            add_one(ot0, dbc0, 0, 6, True)
            add_one(ot0, dbc0, 0, 7, False)
            nc.sync.dma_start(out=out_v[0, :, 4:8, :], in_=ot0[:, 4:8, :])
            dbc_next = bcast(2)
            add_one(ot1, dbc1, 1, 4, True)
            add_one(ot1, dbc1, 1, 5, False)
            add_one(ot1, dbc1, 1, 6, True)
            add_one(ot1, dbc1, 1, 7, False)
            nc.sync.dma_start(out=out_v[1, :, 4:8, :], in_=ot1[:, 4:8, :])

            # steady state: one 2 MB DMA per label; DVE takes 6 of 8 adds,
            # ACT the other 2 (ACT also runs the bcast evacuations);
            # bcast(lb+1) is issued ahead of the adds.
            for lb in range(2, LH):
                dbc = dbc_next
                if lb + 1 < LH:
                    dbc_next = bcast(lb + 1)
                ot = opool.tile([128, ICH, S], F16, tag="ot")
                for ic in range(ICH):
                    add_one(ot, dbc, lb, ic, ic < 6)
                nc.sync.dma_start(out=out_v[lb, :, :, :], in_=ot[:, :, :])
    nc.compile()
    return nc


def kernel(head, dep, label_W, label_b):
    global LAST_RESULTS
    head = np.ascontiguousarray(np.asarray(head, dtype=np.float32))
    dep = np.ascontiguousarray(np.asarray(dep, dtype=np.float32))
    label_W = np.asarray(label_W, dtype=np.float32)
    label_b = np.asarray(label_b, dtype=np.float32)

    headT = np.ascontiguousarray(head.transpose(0, 2, 1))  # [B, D, S]
    depT = np.ascontiguousarray(dep.transpose(0, 2, 1))
    whT = np.ascontiguousarray(label_W[:, :D].T)           # [D, L]
    wdT = np.ascontiguousarray(label_W[:, D:].T)           # [D, L]

    # one-hot selector sel[k, l*128 + p] = (k == l), replicated at
    # partition groups 0 and 32 (one per col-tiled d-score stream)
    sel = np.zeros((48, LH * 128), dtype=np.float32)
    for lb in range(LH):
        sel[lb, lb * 128:(lb + 1) * 128] = 1.0
    sel[32:48] = sel[0:LH]
    # identity blocks for the h transposes at partition groups 0/32/64
    # (the h1 partial-sum transposes read all three)
    id16 = np.zeros((80, LH), dtype=np.float32)
    id16[0:16] = np.eye(LH, dtype=np.float32)
    id16[32:48] = np.eye(LH, dtype=np.float32)
    id16[64:80] = np.eye(LH, dtype=np.float32)

    in_maps = []
    for c in range(NCORES):
        b, lh = divmod(c, 2)
        ls = slice(lh * LH, (lh + 1) * LH)
        bc = np.zeros((48, 1), dtype=np.float32)
        bc[0:LH, 0] = label_b[ls]
        bc[32:48, 0] = label_b[ls]
        in_maps.append({
            "headT": headT[b],
            "depT": depT[b],
            "whT": np.ascontiguousarray(whT[:, ls]),
            "wdT": np.ascontiguousarray(wdT[:, ls]),
            "bcol": bc,
            "sel": sel,
            "id16": id16,
        })

    if "nc" not in _CACHE:
        _CACHE["nc"] = _build()
    nc = _CACHE["nc"]

    res = run_bass_kernel_spmd(nc, in_maps, core_ids=list(range(NCORES)),
                               trace=TRACE, trace_cores=TRACE_CORES)
    LAST_RESULTS = res

    out = np.empty((B, L, S, S), dtype=np.float32)
    for c in range(NCORES):
        b, lh = divmod(c, 2)
        # device layout [l, p, c, j] with i = c*128 + p -> [l, i, j]
        o = np.asarray(res.results[c]["out"])  # [16, 128, 8, 1024] f16
        o = o.transpose(0, 2, 1, 3).reshape(LH, S, S)
        out[b, lh * LH:(lh + 1) * LH] = o.astype(np.float32)
    return out
